# revision 8
# baseline (speedup 1.0000x reference)
"""AntiBurst kernel for Trainium2 — data-parallel over batch on 8 NeuronCores.

Reference (per batch b, x: (B=32, D=768, H=32, W=32), N = H*W = 1024):
    xf  = x[b].reshape(D, N)
    G   = xf.T @ xf
    dis = (-2 + 2*G)*ab_w + ab_b
    w   = sum_m sigmoid(dis[:, m])
    out = xf / w**ab_p

Device kernel (per core, 4 batches):
  - Gram matrix via TensorE in bf16 (host pre-casts x), f32 PSUM accum.
    Only the upper-triangular block strips are computed (G is symmetric):
    row-block j covers columns j*128..N. 56% of the full matmul work.
  - sum_m sigmoid(z) = N/2 + 0.5*sum_m tanh(z/2): tanh + row-sum fused in one
    ScalarE activation (accum_out), scale/bias from runtime ab_params.
    Tanh and Exp share one activation-table set, so the table loads once.
  - The missing lower-triangle row-sum contributions are column sums of the
    strips (symmetry): a running strip-sum S is accumulated on VectorE, then
    one ones-vector matmul per block produces the column sums in [128,1]
    layout directly.
  - w^-p = exp(-p*ln(w)): log2(w) computed on VectorE with exponent/mantissa
    bit tricks (deg-3 polynomial), Exp on ScalarE. No extra table set.
  - r = w^-p is transposed with VectorE 32x32 block transposes, gathered to a
    single row via SBUF->SBUF DMA, broadcast to all 128 partitions with a
    K=1 ones-matmul on TensorE, and multiplied into x on VectorE (bf16).
"""
import numpy as np
import ml_dtypes

import concourse.bass as bass
import concourse.mybir as mybir
import concourse.tile as tile
from concourse import bacc
from concourse.bass_utils import run_bass_kernel_spmd

F32 = mybir.dt.float32
I32 = mybir.dt.int32
BF16 = mybir.dt.bfloat16
B, D, H, W = 32, 768, 32, 32
N = H * W
NCORES = 8
BPC = B // NCORES
DC = D // 128
NJ = N // 128

# deg-3 fit of log2(m) on [1,2), in ((m+A)*m + B)*m*C3 + C0 form
_m = np.linspace(1.0, 2.0, 4097)
_C = np.polyfit(_m, np.log2(_m), 3)
C3, C2, C1, C0 = [float(c) for c in _C]
A_ = C2 / C3
B_ = C1 / C3
LN2 = float(np.log(2.0))


def build(iters: int = 1, dyn: int = 0):
    nc = bacc.Bacc("TRN2", target_bir_lowering=False, debug=False, num_devices=NCORES)
    x_d = nc.dram_tensor("x", [BPC, DC, 128, N], BF16, kind="ExternalInput").ap()
    ab_d = nc.dram_tensor("ab_params", [1, 3], F32, kind="ExternalInput").ap()
    out_d = nc.dram_tensor("out", [BPC, DC, 128, N], BF16, kind="ExternalOutput").ap()

    with tile.TileContext(nc) as tc:
        with (
            tc.tile_pool(name="const", bufs=1) as constp,
            tc.tile_pool(name="x", bufs=18) as xpool,
            tc.tile_pool(name="sig", bufs=8) as sigpool,
            tc.tile_pool(name="wsm", bufs=3) as wpool,
            tc.tile_pool(name="rbc", bufs=2) as rbcpool,
            tc.tile_pool(name="ob", bufs=8) as opool,
            tc.tile_pool(name="psg", bufs=2, space="PSUM") as psgpool,
            tc.tile_pool(name="psc", bufs=2, space="PSUM") as pscpool,
            tc.tile_pool(name="psb", bufs=2, space="PSUM") as psbpool,
            tc.tile_pool(name="S", bufs=2) as spool,
        ):
            # runtime scalars: tanh arg = (s*G + t)/2 with s = 2 ab_w,
            # t = ab_b - 2 ab_w  ->  scale = ab_w, bias = ab_b/2 - ab_w
            ab_sb = constp.tile([128, 3], F32)
            nc.sync.dma_start(ab_sb[:], ab_d.to_broadcast((128, 3)))
            s_half = constp.tile([128, 1], F32)
            nc.vector.tensor_copy(s_half[:], ab_sb[:, 0:1])
            t_half = constp.tile([128, 1], F32)
            nc.vector.tensor_scalar(t_half[:], ab_sb[:, 1:2], 0.5, None,
                                    mybir.AluOpType.mult)
            nc.vector.tensor_sub(t_half[:], t_half[:], ab_sb[:, 0:1])
            negp_ln2 = constp.tile([128, 1], F32)
            nc.vector.tensor_scalar_mul(negp_ln2[:], ab_sb[:, 2:3], -LN2)
            ones_col = constp.tile([128, 1], BF16)
            nc.vector.memset(ones_col[:], 1.0)
            ones_bf_row = constp.tile([1, 128], BF16)
            nc.vector.memset(ones_bf_row[:], 1.0)
            warm_row = constp.tile([1, 512], BF16)
            nc.vector.memset(warm_row[:], 1.0)
            for _wu in range(10):
                psW = psbpool.tile([128, 512], F32, tag="psB")
                nc.tensor.matmul(psW[:], ones_bf_row[:], warm_row[:],
                                 start=True, stop=True, skip_group_check=True)

            def emit_loads(b):
                xc = []
                for c in range(DC):
                    t = xpool.tile([128, N], BF16, tag="x")
                    nc.sync.dma_start(t[:], x_d[b, c])
                    xc.append(t)
                return xc

            def emit_strip(b, j, xc, acc):
                wdt = N - j * 128
                psG = psgpool.tile([128, N], F32, tag="psG")
                for c in range(DC):
                    lhsT = xc[c][:, j * 128:(j + 1) * 128]
                    o = 0
                    while o < wdt:
                        nn = min(512, wdt - o)
                        nc.tensor.matmul(
                            psG[:, o:o + nn],
                            lhsT,
                            xc[c][:, j * 128 + o:j * 128 + o + nn],
                            start=(c == 0),
                            stop=(c == DC - 1),
                        )
                        o += nn
                sg = sigpool.tile([128, N], BF16, tag="sg")
                nc.scalar.activation(
                    sg[:, 0:wdt], psG[:, 0:wdt],
                    mybir.ActivationFunctionType.Tanh,
                    bias=t_half[:], scale=s_half[:],
                    accum_out=acc[:, j:j + 1],
                )
                return sg

            def emit_saccum(j, sg, S):
                if j == 0:
                    nc.vector.tensor_copy(S[:], sg[:, 128:N])
                elif j < NJ - 1:
                    nc.vector.tensor_add(
                        S[:, j * 128:N - 128],
                        S[:, j * 128:N - 128],
                        sg[:, 128:(NJ - j) * 128],
                    )

            def emit_colsum(mb, S, wcols):
                # one-shot column sum; each wcols column is written exactly
                # once (start=True clears has_written for the whole bank, so
                # no accumulation groups may interleave in this bank)
                nc.tensor.matmul(
                    wcols[:, mb:mb + 1],
                    S[:, (mb - 1) * 128:mb * 128],
                    ones_col[:],
                    start=True, stop=True, skip_group_check=True,
                )

            NH = NJ // 2

            def emit_rchain_half(h, acc, wcols, r_bc):
                """log2/exp/transpose/gather/broadcast for j-blocks h*4..h*4+3."""
                hs = slice(h * NH, (h + 1) * NH)
                tg = str(h)
                tot = wpool.tile([128, NH], F32, tag="tot" + tg)
                if h == 0:
                    nc.vector.tensor_copy(tot[:, 0:1], acc[:, 0:1])
                    nc.vector.tensor_add(tot[:, 1:NH], acc[:, 1:NH],
                                         wcols[:, 1:NH])
                else:
                    nc.vector.tensor_add(tot[:], acc[:, hs], wcols[:, hs])
                w2 = wpool.tile([128, NH], F32, tag="w2" + tg)
                nc.vector.tensor_scalar_add(w2[:], tot[:], float(N))
                iw = w2[:].bitcast(I32)
                e_i = wpool.tile([128, NH], I32, tag="ei" + tg)
                nc.vector.tensor_scalar(
                    e_i[:], iw, 23, None, mybir.AluOpType.arith_shift_right
                )
                e_f = wpool.tile([128, NH], F32, tag="ef" + tg)
                nc.vector.tensor_copy(e_f[:], e_i[:])
                m_i = wpool.tile([128, NH], I32, tag="mi" + tg)
                nc.vector.tensor_scalar(
                    m_i[:], iw, 0x007FFFFF, 0x3F800000,
                    mybir.AluOpType.bitwise_and, mybir.AluOpType.bitwise_or,
                )
                m_f = m_i[:].bitcast(F32)
                u = wpool.tile([128, NH], F32, tag="u" + tg)
                nc.vector.scalar_tensor_tensor(
                    u[:], m_f, A_, m_f,
                    op0=mybir.AluOpType.add, op1=mybir.AluOpType.mult,
                )
                v = wpool.tile([128, NH], F32, tag="v" + tg)
                nc.vector.scalar_tensor_tensor(
                    v[:], u[:], B_, m_f,
                    op0=mybir.AluOpType.add, op1=mybir.AluOpType.mult,
                )
                pre = wpool.tile([128, NH], F32, tag="pre" + tg)
                nc.vector.tensor_scalar(
                    pre[:], v[:], C3, C0 - 128.0,
                    mybir.AluOpType.mult, mybir.AluOpType.add,
                )
                lg = wpool.tile([128, NH], F32, tag="lg" + tg)
                nc.vector.tensor_add(lg[:], pre[:], e_f[:])
                r_pad = wpool.tile([128, 32], BF16, tag="rpad" + tg)
                nc.vector.memset(r_pad[:, NH:32], 0.0)
                nc.scalar.activation(
                    r_pad[:, 0:NH], lg[:], mybir.ActivationFunctionType.Exp,
                    scale=negp_ln2[:],
                )
                r_t = wpool.tile([32, 128], BF16, tag="rt" + tg)
                for q in range(4):
                    nc.vector.transpose(
                        r_t[0:32, q * 32:(q + 1) * 32],
                        r_pad[q * 32:(q + 1) * 32, 0:32],
                    )
                r_row = wpool.tile([1, N // 2], BF16, tag="rrow" + tg)
                nc.sync.dma_start(
                    r_row[:].rearrange("o (j p) -> o j p", p=128),
                    r_t[0:NH, :],
                )
                psB = psbpool.tile([128, 512], F32, tag="psB")
                nc.tensor.matmul(psB[:], ones_bf_row[:], r_row[:])
                nc.vector.tensor_copy(r_bc[:, h * 512:(h + 1) * 512], psB[:])

            def emit_mults_h0(xc, r_bc):
                obfs = []
                for c in range(DC):
                    obf = opool.tile([128, N], BF16, tag="ob")
                    nc.vector.tensor_mul(obf[:, 0:512], xc[c][:, 0:512],
                                         r_bc[:, 0:512])
                    obfs.append(obf)
                return obfs

            def emit_mults_h1(b, xc, r_bc, obfs):
                for c in range(DC):
                    nc.vector.tensor_mul(obfs[c][:, 512:N], xc[c][:, 512:N],
                                         r_bc[:, 512:N])
                    nc.sync.dma_start(out_d[b, c], obfs[c][:])

            def emit_iter():
                pending = None
                for b in range(BPC):
                    xc = emit_loads(b)
                    acc = wpool.tile([128, NJ], F32, tag="acc")
                    wcols = pscpool.tile([128, NJ], F32, tag="wcols")
                    S = spool.tile([128, N - 128], BF16, tag="S")
                    r_bc = rbcpool.tile([128, N], BF16)
                    sg_prev = emit_strip(b, 0, xc, acc)
                    if pending is not None:
                        pb, pxc, pacc, pwcols, prbc, pobfs = pending
                        emit_rchain_half(1, pacc, pwcols, prbc)
                        emit_mults_h1(pb, pxc, prbc, pobfs)
                    obfs = None
                    for j in range(1, NJ):
                        sg_j = emit_strip(b, j, xc, acc)
                        emit_saccum(j - 1, sg_prev, S)
                        if j >= 2:
                            emit_colsum(j - 1, S, wcols)
                        if j == 5:
                            emit_rchain_half(0, acc, wcols, r_bc)
                            obfs = emit_mults_h0(xc, r_bc)
                        sg_prev = sg_j
                    emit_colsum(NJ - 1, S, wcols)
                    pending = (b, xc, acc, wcols, r_bc, obfs)
                pb, pxc, pacc, pwcols, prbc, pobfs = pending
                emit_rchain_half(1, pacc, pwcols, prbc)
                emit_mults_h1(pb, pxc, prbc, pobfs)

            if dyn:
                with tc.For_i(0, dyn, 1):
                    emit_iter()
            else:
                for _ in range(iters):
                    emit_iter()
    nc.compile()
    return nc


_CACHE: dict = {}


def _get_nc(iters: int = 1):
    if iters not in _CACHE:
        _CACHE[iters] = build(iters)
    return _CACHE[iters]


def run_sharded(xbf_shards, ab2d, iters: int = 1):
    nc = _get_nc(iters)
    in_maps = [{"x": xbf_shards[i], "ab_params": ab2d} for i in range(NCORES)]
    res = run_bass_kernel_spmd(nc, in_maps, core_ids=list(range(NCORES)))
    return [res.results[i]["out"] for i in range(NCORES)], res


def kernel(x: np.ndarray, ab_params: np.ndarray) -> np.ndarray:
    assert x.shape == (B, D, H, W), f"unexpected x shape {x.shape}"
    xf = np.ascontiguousarray(np.asarray(x, dtype=np.float32).reshape(B, DC, 128, N))
    xbf = xf.astype(ml_dtypes.bfloat16)
    ab2d = np.ascontiguousarray(
        np.asarray(ab_params, dtype=np.float32).reshape(1, 3))
    shards = [xbf[i * BPC:(i + 1) * BPC] for i in range(NCORES)]
    outs, _ = run_sharded(shards, ab2d)
    out = np.concatenate(outs, axis=0).astype(np.float32)
    return out.reshape(B, D, H, W)
